# revision 38
# baseline (speedup 1.0000x reference)
"""Trainium2 Bass kernel for windowed mean-pooling (segment_reduce).

Computes, for each (batch b, window w):
    out[b, w, :] = mean over t in [begins[b,w], ends'[b,w]) of features[b, t, :]
where ends' = clip(ends, begins, begins + 8) (the reference gathers at most
MAX_WINDOW=8 tokens) and empty windows produce 0 (count clamped to >= 1).

Strategy (data-parallel over batch, one sample per NeuronCore):
  - Windows are mean-pooled via mask matmuls: out_slot = M^T @ F with M a
    host-built fp8 0/1 strip, F the slot's tokens, fp32 PSUM accumulate.
  - Windows are assigned to SLOTS greedily in sorted-begin order: a slot
    takes up to 128 windows as long as the union of their token intervals
    fits in 256 tokens.  The slot's tokens are RE-PACKED on host into
    exactly 2 aligned K-tiles, so every slot costs exactly 2 (slot, K-tile)
    mask matmul pairs (~33 total vs ~49 for global aligned packing); slots
    whose tokens fit one tile (the runt) cost a single pair.
  - The kernel is HBM-byte-bound (~5.3 MB/core ~ 15 us at 358 GB/s), so
    bytes are minimized hard: features fp8 E3M4 (~3.2 MB incl. repack
    padding), masks fp8 with the tile-0 strip's zero column-tail trimmed
    (the full-width tile-1 strip runs FIRST so its start=True write
    has_written-covers every PSUM row; the trimmed strip then accumulates),
    outputs fp8 in a partition-major [P, ns*D] layout (contiguous multi-KB
    descriptors), fp32 1/count scales padded to 512 B/partition.
  - PSUM evacuation applies the per-window 1/count scale, split 384+384
    across ScalarE ((N+352)/1.2 ns) and VectorE ((N+~210)/0.96 ns), which
    balance at that split.  PSUM tiles are [P, 1024] = exactly 2 banks so
    no two in-flight slots share a bank (a shared bank serializes the DVE
    read of slot s against slot s+1's PE writes); 4 bufs fill all 8 banks.
  - Startup: warm-up matmuls hold the PE's HAM activity window open while
    the first mask strip (SP HWDGE ring) and feature chunks (GPSIMD SWDGE,
    growing chunk sizes) land; the warm-up source comes from a VectorE
    memset so GpSimd's first op is SWDGE descriptor generation.  Putting
    feature chunks on the SP ring ahead of the masks measured consistently
    worse -- masks gate the same matmuls and the SP ring serializes.
"""

import os
import sys

import numpy as np

for _p in ("/opt/trn_rl_repo", "/root/.axon_site/_ro/trn_rl_repo"):
    if os.path.isdir(_p) and _p not in sys.path:
        sys.path.insert(0, _p)

from concourse import bacc, mybir  # noqa: E402
import concourse.tile as tile  # noqa: E402
from concourse.bass_utils import run_bass_kernel_spmd  # noqa: E402

B, T, D, W = 8, 4096, 768, 2048
MAXWIN = 8
P = 128
SLOT_TOK = 256  # tokens per slot (2 K-tiles, repacked)
N_WARM = 5  # PE p-state warm-up matmuls bridging until the first DMAs land
IVW = 128  # iv tensor width (>= ns; 512 B/partition keeps DMA descriptors at line rate)
F32 = mybir.dt.float32
FP8 = mybir.dt.float8e3
NP_FP8 = mybir.dt.np(mybir.dt.float8e3)


def _fchunks(n, sizes0=(2, 4, 6), big=8):
    """Chunk sizes, small first so early slots' data lands first."""
    sizes = []
    for s in sizes0:
        if sum(sizes) + s > n:
            break
        sizes.append(s)
    rem = n - sum(sizes)
    while rem > 0:
        take = min(big, rem)
        sizes.append(take)
        rem -= take
    assert sum(sizes) == n and all(s > 0 for s in sizes), (sizes, n)
    return sizes


def _ogroups(ns):
    """Output DMA slot groups; small tail groups shorten the drain."""
    groups = []
    rem = ns
    while rem > 3:
        take = min(5, rem - 2)
        groups.append(take)
        rem -= take
    if rem > 1:
        groups.append(rem - 1)
        rem = 1
    groups.append(1)
    assert sum(groups) == ns
    return groups


def _build_program(ns, nkt, pairs, m_total):
    """pairs: per slot, list of (mask col base, width, psum row offset,
    feature tile index)."""
    nc = bacc.Bacc(None)

    fhi_d = nc.declare_dram_parameter("fhi", [P, nkt, D], FP8, isOutput=False)
    mask_d = nc.declare_dram_parameter("mask", [P, m_total], FP8, isOutput=False)
    iv_d = nc.declare_dram_parameter("iv", [P, IVW], F32, isOutput=False)
    out_d = nc.declare_dram_parameter("out", [P, ns * D], FP8, isOutput=True)

    # Feature chunks stream via GPSIMD SWDGE (growing sizes keep arrival
    # smooth just ahead of the DMA-paced slot cadence); masks + iv ride the
    # SP HWDGE ring.
    # Fine-grained chunks: a chunk's completion receipt (~1.5-3 us) fires
    # only after the WHOLE chunk lands, so big chunks make slot data arrive
    # in lumps the PE stalls on (1.9 us stalls seen with 6-8-tile chunks).
    # GpSimd's Q7 has plenty of slack for the extra descriptor generation.
    fchunks = _fchunks(nkt, sizes0=(1, 1, 3), big=4)
    nsync = 0  # features all via SWDGE: measured faster than any mix that
    # puts feature chunks ahead of the mask strips on the SP ring
    # Mask chunk cuts in columns, aligned to slot boundaries (~4/10/10/10
    # pair-equivalents); slot s's strips must be in the chunk covering them.
    sbounds = np.cumsum([0] + [sum(w for _, w, _, _ in ps_) for ps_ in pairs])
    scuts = [0] + [min(x, ns) for x in (2, 8)] + [ns]
    scuts = sorted(set(scuts))
    mchunks = [
        (sbounds[scuts[i]], sbounds[scuts[i + 1]])
        for i in range(len(scuts) - 1)
        if sbounds[scuts[i]] < sbounds[scuts[i + 1]]
    ]
    ogroups = _ogroups(ns)

    with tile.TileContext(nc) as tc:
        with (
            tc.tile_pool(name="ivp", bufs=1) as iv_pool,
            tc.tile_pool(name="warm", bufs=1) as warm_pool,
            tc.tile_pool(name="fslab", bufs=1) as f_pool,
            tc.tile_pool(name="mslab", bufs=1) as m_pool,
            tc.tile_pool(name="outp", bufs=1) as out_pool,
            tc.tile_pool(name="psum", bufs=4, space="PSUM") as psum_pool,
        ):
            # Warm-up source via VectorE so GpSimd's first op is the SWDGE
            # descriptor generation for feature chunk 1.
            wsrc = warm_pool.tile([P, 512], FP8)
            nc.vector.memset(wsrc[:], 0.25)

            # SP HWDGE ring order: first mask strip, iv, remaining mask
            # strips.  SWDGE (Q7) concurrently generates + streams the
            # feature chunks.
            mask_sb = m_pool.tile([P, m_total], FP8)
            nc.sync.dma_start(
                out=mask_sb[:, mchunks[0][0] : mchunks[0][1]],
                in_=mask_d[:, mchunks[0][0] : mchunks[0][1]],
            )
            iv_sb = iv_pool.tile([P, IVW], F32)
            nc.sync.dma_start(out=iv_sb[:], in_=iv_d[:])
            fhi_tiles = []
            k2chunk = []
            k0 = 0
            for j, sz in enumerate(fchunks):
                fh = f_pool.tile([P, sz, D], FP8, name=f"fh{j}", tag=f"fh{j}")
                eng = nc.sync if j < nsync else nc.gpsimd
                eng.dma_start(out=fh[:], in_=fhi_d[:, k0 : k0 + sz, :])
                fhi_tiles.append(fh)
                for s in range(sz):
                    k2chunk.append((j, s))
                k0 += sz
            assert k0 == nkt

            # Remaining mask strips.
            for lo, hi in mchunks[1:]:
                nc.sync.dma_start(
                    out=mask_sb[:, lo:hi], in_=mask_d[:, lo:hi]
                )

            # PE p-state warm-up: keep the PE busy until the first slot's
            # mask + feature DMAs land.
            wps = psum_pool.tile([P, 1024], F32, name="warm", tag="ps")
            for _ in range(N_WARM):
                nc.tensor.matmul(
                    wps[:, 0:512], wsrc[:, 0:P], wsrc[:], start=True, stop=True
                )

            os_slab = out_pool.tile([P, ns, D], FP8)
            gcuts = np.cumsum([0] + ogroups)
            gi = 0
            for s in range(ns):
                # [P, 1024] = exactly 2 PSUM banks per buf; 4 bufs fill all
                # 8 banks and give the evacuation two slots of slack.
                ps = psum_pool.tile([P, 1024], F32, name=f"ps{s}", tag="ps")
                np_s = len(pairs[s])
                for j, (cb, w, moff, kt) in enumerate(pairs[s]):
                    lh = mask_sb[:, cb : cb + w]
                    cj, cs = k2chunk[kt]
                    rh = fhi_tiles[cj][:, cs, :]
                    first = j == 0
                    last = j == np_s - 1
                    for n0, nn in ((0, 512), (512, 256)):
                        nc.tensor.matmul(
                            ps[moff : moff + w, n0 : n0 + nn],
                            lh, rh[:, n0 : n0 + nn],
                            start=first, stop=(last and n0 == 512),
                        )
                # PSUM evacuation with the 1/count scale; 384+384 balances
                # ScalarE (0.83 ns/el + 293 ns) vs VectorE (1.04 + 217).
                nc.scalar.mul(
                    out=os_slab[:, s, 0:384], in_=ps[:, 0:384],
                    mul=iv_sb[:, s : s + 1],
                )
                nc.vector.tensor_scalar(
                    os_slab[:, s, 384:D], ps[:, 384:D],
                    iv_sb[:, s : s + 1], None, mybir.AluOpType.mult,
                )
                if s == gcuts[gi + 1] - 1:
                    g0, g1 = gcuts[gi], gcuts[gi + 1]
                    nc.sync.dma_start(
                        out=out_d[:, g0 * D : g1 * D],
                        in_=os_slab[:, g0:g1, :],
                    )
                    gi += 1

    nc.finalize()
    return nc


def _assign_slots(b, e_eff):
    """Per-core greedy slot assignment in sorted-begin order: a slot takes
    up to 128 windows whose token-interval union stays <= SLOT_TOK tokens.

    Returns (ns, slot_of[B,W], pos_of[B,W], slot_tokens[c][s] -> np.array).
    """
    slot_of = np.full((B, W), -1, np.int32)
    pos_of = np.full((B, W), -1, np.int32)
    slot_tokens = []
    ns = 0
    for c in range(B):
        order = np.argsort(b[c], kind="stable")
        bs, es = b[c][order], e_eff[c][order]
        toks_c = []
        i = 0
        while i < W:
            covered = 0
            cur_end = int(bs[i])
            ivals = []
            j = i
            while j < W and j - i < P:
                nb, ne = int(bs[j]), int(es[j])
                add = max(0, ne - max(nb, cur_end))
                if covered + add > SLOT_TOK:
                    break
                covered += add
                if ne > cur_end:
                    ivals.append((max(nb, cur_end), ne))
                    cur_end = ne
                j += 1
            s = len(toks_c)
            slot_of[c, order[i:j]] = s
            pos_of[c, order[i:j]] = np.arange(j - i)
            toks_c.append(
                np.concatenate([np.arange(a, z) for a, z in ivals])
                if ivals
                else np.zeros(0, np.int64)
            )
            i = j
        slot_tokens.append(toks_c)
        ns = max(ns, len(toks_c))
    return ns, slot_of, pos_of, slot_tokens


def _prepare(features, begins, ends):
    feats = np.asarray(features, dtype=np.float32)
    assert feats.shape == (B, T, D), feats.shape
    b = np.clip(np.asarray(begins).astype(np.int64), 0, T - 1)
    e = np.asarray(ends).astype(np.int64)
    # Reference gathers at most MAXWIN tokens starting at b; empty -> count 1.
    e_eff = np.clip(e, b, np.minimum(b + MAXWIN, T))
    counts = np.maximum(e_eff - b, 1).astype(np.float32)
    inv = (1.0 / counts).astype(np.float32)

    ns, slot_of, pos_of, slot_tokens = _assign_slots(b, e_eff)
    assert ns <= IVW, ns

    # Joint (cross-core max) pair structure per slot.  Windows are sorted by
    # begin, so windows starting in tile 0 are a position-prefix [0, c1) and
    # windows reaching into tile 1 are a position-suffix [c0, 128), c0 <= c1.
    # Ship only mask columns [0, c1r) for pair 0 and [c0r, 128) for pair 1
    # (32-aligned); the two matmuls' M-ranges still cover every PSUM row.
    lbs = [[None] * ns for _ in range(B)]
    les = [[None] * ns for _ in range(B)]
    c1s = np.zeros(ns, np.int64)
    c0s = np.full(ns, P, np.int64)
    two = np.zeros(ns, bool)  # slot has any tile-1 tokens on any core
    for c in range(B):
        for s, toks in enumerate(slot_tokens[c]):
            ws = np.nonzero(slot_of[c] == s)[0]
            if not len(ws):
                continue
            o = np.argsort(pos_of[c, ws])
            ws = ws[o]
            lb = np.searchsorted(toks, b[c, ws])
            le = lb + (e_eff[c, ws] - b[c, ws])
            lbs[c][s], les[c][s] = lb, le
            c1s[s] = max(c1s[s], int((lb < P).sum()))
            over = np.nonzero(le > P)[0]
            if len(over):
                c0s[s] = min(c0s[s], int(over[0]))
            if len(toks) > P:
                two[s] = True
    # A matmul PSUM write with a nonzero partition offset may span at most
    # one 32-partition col-group (walrus birverifier), so only pair 0's
    # column TAIL is trimmed (its windows are a position-prefix).  The
    # full-width tile-1 pair runs FIRST in each slot: its start=True write
    # covers (and has_written-clears) every PSUM row, and the trimmed
    # tile-0 pair then accumulates on rows [0, c1r) only.
    pairs = []
    kt = 0
    col = 0
    for s in range(ns):
        if two[s]:
            c1r = min(P, -(-int(c1s[s]) // 32) * 32)
            pairs.append([(col, P, 0, kt + 1), (col + P, c1r, 0, kt)])
            col += P + c1r
            kt += 2
        else:
            pairs.append([(col, P, 0, kt)])  # full width: writes every row
            col += P
            kt += 1
    nkt, m_total = kt, col

    in_maps = []
    unperm = []
    for c in range(B):
        hi = np.zeros((P, nkt, D), NP_FP8)
        slab = np.zeros((P, m_total), NP_FP8)
        ivm = np.zeros((P, IVW), np.float32)
        ivm[pos_of[c], slot_of[c]] = inv[c]
        for s, toks in enumerate(slot_tokens[c]):
            nt = len(toks)
            ktiles = sorted(pr[3] for pr in pairs[s])
            if nt:
                pk = feats[c, toks].astype(NP_FP8)
                pad = np.zeros((P * len(ktiles), D), NP_FP8)
                pad[:nt] = pk
                hi[:, ktiles, :] = pad.reshape(len(ktiles), P, D).transpose(
                    1, 0, 2
                )
            if lbs[c][s] is None:
                continue
            lb, le = lbs[c][s], les[c][s]
            nw = len(lb)
            for cb, w, moff, pkt in pairs[s]:
                tloc = pkt - ktiles[0]  # which 128-token tile of the slot
                lt = P * tloc + np.arange(P)  # local token row per partition
                sub = (
                    (lb[None, :] <= lt[:, None]) & (lt[:, None] < le[None, :])
                ).astype(NP_FP8)
                # columns for positions [moff, moff+w) of this slot
                pcols = np.arange(moff, min(moff + w, nw))
                slab[:, cb + pcols - moff] = sub[:, pcols]
        in_maps.append({"fhi": hi, "mask": slab, "iv": ivm})
        unperm.append((slot_of[c].astype(np.int64), pos_of[c].astype(np.int64)))
    return ns, nkt, pairs, m_total, in_maps, unperm


def run(features, begins, ends, trace=False):
    """Build + run on 8 NeuronCores; returns (output, BassKernelResults)."""
    ns, nkt, pairs, m_total, in_maps, unperm = _prepare(features, begins, ends)
    nc = _build_program(ns, nkt, pairs, m_total)
    res = run_bass_kernel_spmd(nc, in_maps, list(range(B)), trace=trace)
    out = np.stack(
        [
            res.results[c]["out"]
            .reshape(P, ns, D)[unperm[c][1], unperm[c][0]]
            .astype(np.float32)
            for c in range(B)
        ],
        axis=0,
    )
    return out, res


def kernel(features, begins, ends):
    out, _ = run(features, begins, ends, trace=False)
    return out


# revision 39
# speedup vs baseline: 1.1094x; 1.1094x over previous
"""Trainium2 Bass kernel for windowed mean-pooling (segment_reduce).

Computes, for each (batch b, window w):
    out[b, w, :] = mean over t in [begins[b,w], ends'[b,w]) of features[b, t, :]
where ends' = clip(ends, begins, begins + 8) (the reference gathers at most
MAX_WINDOW=8 tokens) and empty windows produce 0 (count clamped to >= 1).

Strategy (data-parallel over batch, one sample per NeuronCore):
  - Windows are mean-pooled via mask matmuls: out_slot = M^T @ F with M a
    host-built fp8 0/1 strip, F the slot's tokens, fp32 PSUM accumulate.
  - Windows are assigned to SLOTS greedily in sorted-begin order: a slot
    takes up to 128 windows as long as the union of their token intervals
    fits in 256 tokens.  The slot's tokens are RE-PACKED on host into
    exactly 2 aligned K-tiles, so every slot costs exactly 2 (slot, K-tile)
    mask matmul pairs (~33 total vs ~49 for global aligned packing); slots
    whose tokens fit one tile (the runt) cost a single pair.
  - The kernel is HBM-byte-bound (~5.3 MB/core ~ 15 us at 358 GB/s), so
    bytes are minimized hard: features fp8 E3M4 (~3.2 MB incl. repack
    padding), masks fp8 with the tile-0 strip's zero column-tail trimmed
    (the full-width tile-1 strip runs FIRST so its start=True write
    has_written-covers every PSUM row; the trimmed strip then accumulates),
    outputs fp8 in a partition-major [P, ns*D] layout (contiguous multi-KB
    descriptors), fp32 1/count scales padded to 512 B/partition.
  - PSUM evacuation applies the per-window 1/count scale, split 384+384
    across ScalarE ((N+352)/1.2 ns) and VectorE ((N+~210)/0.96 ns), which
    balance at that split.  PSUM tiles are [P, 1024] = exactly 2 banks so
    no two in-flight slots share a bank (a shared bank serializes the DVE
    read of slot s against slot s+1's PE writes); 4 bufs fill all 8 banks.
  - Startup: warm-up matmuls hold the PE's HAM activity window open while
    the first mask strip (SP HWDGE ring) and feature chunks (GPSIMD SWDGE,
    growing chunk sizes) land; the warm-up source comes from a VectorE
    memset so GpSimd's first op is SWDGE descriptor generation.  Putting
    feature chunks on the SP ring ahead of the masks measured consistently
    worse -- masks gate the same matmuls and the SP ring serializes.
"""

import os
import sys

import numpy as np

for _p in ("/opt/trn_rl_repo", "/root/.axon_site/_ro/trn_rl_repo"):
    if os.path.isdir(_p) and _p not in sys.path:
        sys.path.insert(0, _p)

from concourse import bacc, mybir  # noqa: E402
import concourse.tile as tile  # noqa: E402
from concourse.bass_utils import run_bass_kernel_spmd  # noqa: E402

B, T, D, W = 8, 4096, 768, 2048
MAXWIN = 8
P = 128
SLOT_TOK = 256  # tokens per slot (2 K-tiles, repacked)
N_WARM = 5  # PE p-state warm-up matmuls bridging until the first DMAs land
IVW = 128  # iv tensor width (>= ns; 512 B/partition keeps DMA descriptors at line rate)
F32 = mybir.dt.float32
FP8 = mybir.dt.float8e3
NP_FP8 = mybir.dt.np(mybir.dt.float8e3)


def _fchunks(n, sizes0=(2, 4, 6), big=8):
    """Chunk sizes, small first so early slots' data lands first."""
    sizes = []
    for s in sizes0:
        if sum(sizes) + s > n:
            break
        sizes.append(s)
    rem = n - sum(sizes)
    while rem > 0:
        take = min(big, rem)
        sizes.append(take)
        rem -= take
    assert sum(sizes) == n and all(s > 0 for s in sizes), (sizes, n)
    return sizes


def _ogroups(ns):
    """Output DMA slot groups; small tail groups shorten the drain."""
    groups = []
    rem = ns
    while rem > 3:
        take = min(5, rem - 2)
        groups.append(take)
        rem -= take
    if rem > 1:
        groups.append(rem - 1)
        rem = 1
    groups.append(1)
    assert sum(groups) == ns
    return groups


def _build_program(ns, nkt, pairs, m_total):
    """pairs: per slot, list of (mask col base, width, psum row offset,
    feature tile index)."""
    nc = bacc.Bacc(None)

    fhi_d = nc.declare_dram_parameter("fhi", [P, nkt, D], FP8, isOutput=False)
    mask_d = nc.declare_dram_parameter("mask", [P, m_total], FP8, isOutput=False)
    iv_d = nc.declare_dram_parameter("iv", [P, IVW], F32, isOutput=False)
    out_d = nc.declare_dram_parameter("out", [P, ns * D], FP8, isOutput=True)

    # Feature chunks stream via GPSIMD SWDGE (growing sizes keep arrival
    # smooth just ahead of the DMA-paced slot cadence); masks + iv ride the
    # SP HWDGE ring.
    # Chunk sizes [2,4,6,8,8,...]: coarser chunks measured better than
    # fine-grained ones -- each SWDGE dma_start pays its own ~2-3 us
    # completion receipt, and a deeper SWDGE queue makes receipts worse.
    fchunks = _fchunks(nkt)
    nsync = 0  # features all via SWDGE: measured faster than any mix that
    # puts feature chunks ahead of the mask strips on the SP ring
    # Mask chunk cuts in columns, aligned to slot boundaries (~4/10/10/10
    # pair-equivalents); slot s's strips must be in the chunk covering them.
    sbounds = np.cumsum([0] + [sum(w for _, w, _, _ in ps_) for ps_ in pairs])
    scuts = [0] + [min(x, ns) for x in (2, 8)] + [ns]
    scuts = sorted(set(scuts))
    mchunks = [
        (sbounds[scuts[i]], sbounds[scuts[i + 1]])
        for i in range(len(scuts) - 1)
        if sbounds[scuts[i]] < sbounds[scuts[i + 1]]
    ]
    ogroups = _ogroups(ns)

    with tile.TileContext(nc) as tc:
        with (
            tc.tile_pool(name="ivp", bufs=1) as iv_pool,
            tc.tile_pool(name="warm", bufs=1) as warm_pool,
            tc.tile_pool(name="fslab", bufs=1) as f_pool,
            tc.tile_pool(name="mslab", bufs=1) as m_pool,
            tc.tile_pool(name="outp", bufs=1) as out_pool,
            tc.tile_pool(name="psum", bufs=4, space="PSUM") as psum_pool,
        ):
            # Warm-up source via VectorE so GpSimd's first op is the SWDGE
            # descriptor generation for feature chunk 1.
            wsrc = warm_pool.tile([P, 512], FP8)
            nc.vector.memset(wsrc[:], 0.25)

            # SP HWDGE ring order: first mask strip, iv, remaining mask
            # strips.  SWDGE (Q7) concurrently generates + streams the
            # feature chunks.
            mask_sb = m_pool.tile([P, m_total], FP8)
            nc.sync.dma_start(
                out=mask_sb[:, mchunks[0][0] : mchunks[0][1]],
                in_=mask_d[:, mchunks[0][0] : mchunks[0][1]],
            )
            iv_sb = iv_pool.tile([P, IVW], F32)
            nc.sync.dma_start(out=iv_sb[:], in_=iv_d[:])
            fhi_tiles = []
            k2chunk = []
            k0 = 0
            for j, sz in enumerate(fchunks):
                fh = f_pool.tile([P, sz, D], FP8, name=f"fh{j}", tag=f"fh{j}")
                eng = nc.sync if j < nsync else nc.gpsimd
                eng.dma_start(out=fh[:], in_=fhi_d[:, k0 : k0 + sz, :])
                fhi_tiles.append(fh)
                for s in range(sz):
                    k2chunk.append((j, s))
                k0 += sz
            assert k0 == nkt

            # Remaining mask strips.
            for lo, hi in mchunks[1:]:
                nc.sync.dma_start(
                    out=mask_sb[:, lo:hi], in_=mask_d[:, lo:hi]
                )

            # PE p-state warm-up: keep the PE busy until the first slot's
            # mask + feature DMAs land.
            wps = psum_pool.tile([P, 1024], F32, name="warm", tag="ps")
            for _ in range(N_WARM):
                nc.tensor.matmul(
                    wps[:, 0:512], wsrc[:, 0:P], wsrc[:], start=True, stop=True
                )

            os_slab = out_pool.tile([P, ns, D], FP8)
            gcuts = np.cumsum([0] + ogroups)
            gi = 0
            for s in range(ns):
                # [P, 1024] = exactly 2 PSUM banks per buf; 4 bufs fill all
                # 8 banks and give the evacuation two slots of slack.
                ps = psum_pool.tile([P, 1024], F32, name=f"ps{s}", tag="ps")
                np_s = len(pairs[s])
                for j, (cb, w, moff, kt) in enumerate(pairs[s]):
                    lh = mask_sb[:, cb : cb + w]
                    cj, cs = k2chunk[kt]
                    rh = fhi_tiles[cj][:, cs, :]
                    first = j == 0
                    last = j == np_s - 1
                    for n0, nn in ((0, 512), (512, 256)):
                        nc.tensor.matmul(
                            ps[moff : moff + w, n0 : n0 + nn],
                            lh, rh[:, n0 : n0 + nn],
                            start=first, stop=(last and n0 == 512),
                        )
                # PSUM evacuation with the 1/count scale; 384+384 balances
                # ScalarE (0.83 ns/el + 293 ns) vs VectorE (1.04 + 217).
                nc.scalar.mul(
                    out=os_slab[:, s, 0:384], in_=ps[:, 0:384],
                    mul=iv_sb[:, s : s + 1],
                )
                nc.vector.tensor_scalar(
                    os_slab[:, s, 384:D], ps[:, 384:D],
                    iv_sb[:, s : s + 1], None, mybir.AluOpType.mult,
                )
                if s == gcuts[gi + 1] - 1:
                    g0, g1 = gcuts[gi], gcuts[gi + 1]
                    nc.sync.dma_start(
                        out=out_d[:, g0 * D : g1 * D],
                        in_=os_slab[:, g0:g1, :],
                    )
                    gi += 1

    nc.finalize()
    return nc


def _assign_slots(b, e_eff):
    """Per-core greedy slot assignment in sorted-begin order: a slot takes
    up to 128 windows whose token-interval union stays <= SLOT_TOK tokens.

    Returns (ns, slot_of[B,W], pos_of[B,W], slot_tokens[c][s] -> np.array).
    """
    slot_of = np.full((B, W), -1, np.int32)
    pos_of = np.full((B, W), -1, np.int32)
    slot_tokens = []
    ns = 0
    for c in range(B):
        order = np.argsort(b[c], kind="stable")
        bs, es = b[c][order], e_eff[c][order]
        toks_c = []
        i = 0
        while i < W:
            covered = 0
            cur_end = int(bs[i])
            ivals = []
            j = i
            while j < W and j - i < P:
                nb, ne = int(bs[j]), int(es[j])
                add = max(0, ne - max(nb, cur_end))
                if covered + add > SLOT_TOK:
                    break
                covered += add
                if ne > cur_end:
                    ivals.append((max(nb, cur_end), ne))
                    cur_end = ne
                j += 1
            s = len(toks_c)
            slot_of[c, order[i:j]] = s
            pos_of[c, order[i:j]] = np.arange(j - i)
            toks_c.append(
                np.concatenate([np.arange(a, z) for a, z in ivals])
                if ivals
                else np.zeros(0, np.int64)
            )
            i = j
        slot_tokens.append(toks_c)
        ns = max(ns, len(toks_c))
    return ns, slot_of, pos_of, slot_tokens


def _prepare(features, begins, ends):
    feats = np.asarray(features, dtype=np.float32)
    assert feats.shape == (B, T, D), feats.shape
    b = np.clip(np.asarray(begins).astype(np.int64), 0, T - 1)
    e = np.asarray(ends).astype(np.int64)
    # Reference gathers at most MAXWIN tokens starting at b; empty -> count 1.
    e_eff = np.clip(e, b, np.minimum(b + MAXWIN, T))
    counts = np.maximum(e_eff - b, 1).astype(np.float32)
    inv = (1.0 / counts).astype(np.float32)

    ns, slot_of, pos_of, slot_tokens = _assign_slots(b, e_eff)
    assert ns <= IVW, ns

    # Joint (cross-core max) pair structure per slot.  Windows are sorted by
    # begin, so windows starting in tile 0 are a position-prefix [0, c1) and
    # windows reaching into tile 1 are a position-suffix [c0, 128), c0 <= c1.
    # Ship only mask columns [0, c1r) for pair 0 and [c0r, 128) for pair 1
    # (32-aligned); the two matmuls' M-ranges still cover every PSUM row.
    lbs = [[None] * ns for _ in range(B)]
    les = [[None] * ns for _ in range(B)]
    c1s = np.zeros(ns, np.int64)
    c0s = np.full(ns, P, np.int64)
    two = np.zeros(ns, bool)  # slot has any tile-1 tokens on any core
    for c in range(B):
        for s, toks in enumerate(slot_tokens[c]):
            ws = np.nonzero(slot_of[c] == s)[0]
            if not len(ws):
                continue
            o = np.argsort(pos_of[c, ws])
            ws = ws[o]
            lb = np.searchsorted(toks, b[c, ws])
            le = lb + (e_eff[c, ws] - b[c, ws])
            lbs[c][s], les[c][s] = lb, le
            c1s[s] = max(c1s[s], int((lb < P).sum()))
            over = np.nonzero(le > P)[0]
            if len(over):
                c0s[s] = min(c0s[s], int(over[0]))
            if len(toks) > P:
                two[s] = True
    # A matmul PSUM write with a nonzero partition offset may span at most
    # one 32-partition col-group (walrus birverifier), so only pair 0's
    # column TAIL is trimmed (its windows are a position-prefix).  The
    # full-width tile-1 pair runs FIRST in each slot: its start=True write
    # covers (and has_written-clears) every PSUM row, and the trimmed
    # tile-0 pair then accumulates on rows [0, c1r) only.
    pairs = []
    kt = 0
    col = 0
    for s in range(ns):
        if two[s]:
            c1r = min(P, -(-int(c1s[s]) // 32) * 32)
            pairs.append([(col, P, 0, kt + 1), (col + P, c1r, 0, kt)])
            col += P + c1r
            kt += 2
        else:
            pairs.append([(col, P, 0, kt)])  # full width: writes every row
            col += P
            kt += 1
    nkt, m_total = kt, col

    in_maps = []
    unperm = []
    for c in range(B):
        hi = np.zeros((P, nkt, D), NP_FP8)
        slab = np.zeros((P, m_total), NP_FP8)
        ivm = np.zeros((P, IVW), np.float32)
        ivm[pos_of[c], slot_of[c]] = inv[c]
        for s, toks in enumerate(slot_tokens[c]):
            nt = len(toks)
            ktiles = sorted(pr[3] for pr in pairs[s])
            if nt:
                pk = feats[c, toks].astype(NP_FP8)
                pad = np.zeros((P * len(ktiles), D), NP_FP8)
                pad[:nt] = pk
                hi[:, ktiles, :] = pad.reshape(len(ktiles), P, D).transpose(
                    1, 0, 2
                )
            if lbs[c][s] is None:
                continue
            lb, le = lbs[c][s], les[c][s]
            nw = len(lb)
            for cb, w, moff, pkt in pairs[s]:
                tloc = pkt - ktiles[0]  # which 128-token tile of the slot
                lt = P * tloc + np.arange(P)  # local token row per partition
                sub = (
                    (lb[None, :] <= lt[:, None]) & (lt[:, None] < le[None, :])
                ).astype(NP_FP8)
                # columns for positions [moff, moff+w) of this slot
                pcols = np.arange(moff, min(moff + w, nw))
                slab[:, cb + pcols - moff] = sub[:, pcols]
        in_maps.append({"fhi": hi, "mask": slab, "iv": ivm})
        unperm.append((slot_of[c].astype(np.int64), pos_of[c].astype(np.int64)))
    return ns, nkt, pairs, m_total, in_maps, unperm


def run(features, begins, ends, trace=False):
    """Build + run on 8 NeuronCores; returns (output, BassKernelResults)."""
    ns, nkt, pairs, m_total, in_maps, unperm = _prepare(features, begins, ends)
    nc = _build_program(ns, nkt, pairs, m_total)
    res = run_bass_kernel_spmd(nc, in_maps, list(range(B)), trace=trace)
    out = np.stack(
        [
            res.results[c]["out"]
            .reshape(P, ns, D)[unperm[c][1], unperm[c][0]]
            .astype(np.float32)
            for c in range(B)
        ],
        axis=0,
    )
    return out, res


def kernel(features, begins, ends):
    out, _ = run(features, begins, ends, trace=False)
    return out


# revision 40
# speedup vs baseline: 1.1192x; 1.0089x over previous
"""Trainium2 Bass kernel for windowed mean-pooling (segment_reduce).

Computes, for each (batch b, window w):
    out[b, w, :] = mean over t in [begins[b,w], ends'[b,w]) of features[b, t, :]
where ends' = clip(ends, begins, begins + 8) (the reference gathers at most
MAX_WINDOW=8 tokens) and empty windows produce 0 (count clamped to >= 1).

Strategy (data-parallel over batch, one sample per NeuronCore):
  - Windows are mean-pooled via mask matmuls: out_slot = M^T @ F with M a
    host-built fp8 0/1 strip, F the slot's tokens, fp32 PSUM accumulate.
  - Windows are assigned to SLOTS greedily in sorted-begin order: a slot
    takes up to 128 windows as long as the union of their token intervals
    fits in 256 tokens.  The slot's tokens are RE-PACKED on host into
    exactly 2 aligned K-tiles, so every slot costs exactly 2 (slot, K-tile)
    mask matmul pairs (~33 total vs ~49 for global aligned packing); slots
    whose tokens fit one tile (the runt) cost a single pair.
  - The kernel is HBM-byte-bound (~5.3 MB/core ~ 15 us at 358 GB/s), so
    bytes are minimized hard: features fp8 E3M4 (~3.2 MB incl. repack
    padding), masks fp8 with the tile-0 strip's zero column-tail trimmed
    (the full-width tile-1 strip runs FIRST so its start=True write
    has_written-covers every PSUM row; the trimmed strip then accumulates),
    outputs fp8 in a partition-major [P, ns*D] layout (contiguous multi-KB
    descriptors), fp32 1/count scales padded to 512 B/partition.
  - PSUM evacuation applies the per-window 1/count scale, split 384+384
    across ScalarE ((N+352)/1.2 ns) and VectorE ((N+~210)/0.96 ns), which
    balance at that split.  PSUM tiles are [P, 1024] = exactly 2 banks so
    no two in-flight slots share a bank (a shared bank serializes the DVE
    read of slot s against slot s+1's PE writes); 4 bufs fill all 8 banks.
  - Startup: warm-up matmuls hold the PE's HAM activity window open while
    the first mask strip (SP HWDGE ring) and feature chunks (GPSIMD SWDGE,
    growing chunk sizes) land; the warm-up source comes from a VectorE
    memset so GpSimd's first op is SWDGE descriptor generation.  Putting
    feature chunks on the SP ring ahead of the masks measured consistently
    worse -- masks gate the same matmuls and the SP ring serializes.
"""

import os
import sys

import numpy as np

for _p in ("/opt/trn_rl_repo", "/root/.axon_site/_ro/trn_rl_repo"):
    if os.path.isdir(_p) and _p not in sys.path:
        sys.path.insert(0, _p)

from concourse import bacc, mybir  # noqa: E402
import concourse.tile as tile  # noqa: E402
from concourse.bass_utils import run_bass_kernel_spmd  # noqa: E402

B, T, D, W = 8, 4096, 768, 2048
MAXWIN = 8
P = 128
SLOT_TOK = 256  # tokens per slot (2 K-tiles, repacked)
N_WARM = 6  # PE p-state warm-up matmuls bridging until the first DMAs land
IVW = 128  # iv tensor width (>= ns; 512 B/partition keeps DMA descriptors at line rate)
F32 = mybir.dt.float32
FP8 = mybir.dt.float8e3
NP_FP8 = mybir.dt.np(mybir.dt.float8e3)


def _fchunks(n, sizes0=(2, 4, 6), big=8):
    """Chunk sizes, small first so early slots' data lands first."""
    sizes = []
    for s in sizes0:
        if sum(sizes) + s > n:
            break
        sizes.append(s)
    rem = n - sum(sizes)
    while rem > 0:
        take = min(big, rem)
        sizes.append(take)
        rem -= take
    assert sum(sizes) == n and all(s > 0 for s in sizes), (sizes, n)
    return sizes


def _ogroups(ns):
    """Output DMA slot groups; small tail groups shorten the drain."""
    groups = []
    rem = ns
    while rem > 3:
        take = min(5, rem - 2)
        groups.append(take)
        rem -= take
    if rem > 1:
        groups.append(rem - 1)
        rem = 1
    groups.append(1)
    assert sum(groups) == ns
    return groups


def _build_program(ns, nkt, pairs, m_total, last_rows=P):
    """pairs: per slot, list of (mask col base, width, psum row offset,
    feature tile index).  last_rows: used rows of the final (runt) slot --
    its out DMA ships only those partitions."""
    nc = bacc.Bacc(None)

    fhi_d = nc.declare_dram_parameter("fhi", [P, nkt, D], FP8, isOutput=False)
    mask_d = nc.declare_dram_parameter("mask", [P, m_total], FP8, isOutput=False)
    iv_d = nc.declare_dram_parameter("iv", [P, IVW], F32, isOutput=False)
    out_d = nc.declare_dram_parameter("out", [P, ns * D], FP8, isOutput=True)

    # Feature chunks stream via GPSIMD SWDGE (growing sizes keep arrival
    # smooth just ahead of the DMA-paced slot cadence); masks + iv ride the
    # SP HWDGE ring.
    # Chunk sizes [2,4,6,8,8,...]: coarser chunks measured better than
    # fine-grained ones -- each SWDGE dma_start pays its own ~2-3 us
    # completion receipt, and a deeper SWDGE queue makes receipts worse.
    fchunks = _fchunks(nkt)
    nsync = 0  # features all via SWDGE: measured faster than any mix that
    # puts feature chunks ahead of the mask strips on the SP ring
    # Mask chunk cuts in columns, aligned to slot boundaries (~4/10/10/10
    # pair-equivalents); slot s's strips must be in the chunk covering them.
    sbounds = np.cumsum([0] + [sum(w for _, w, _, _ in ps_) for ps_ in pairs])
    scuts = [0] + [min(x, ns) for x in (2, 8)] + [ns]
    scuts = sorted(set(scuts))
    mchunks = [
        (sbounds[scuts[i]], sbounds[scuts[i + 1]])
        for i in range(len(scuts) - 1)
        if sbounds[scuts[i]] < sbounds[scuts[i + 1]]
    ]
    ogroups = _ogroups(ns)

    with tile.TileContext(nc) as tc:
        with (
            tc.tile_pool(name="ivp", bufs=1) as iv_pool,
            tc.tile_pool(name="warm", bufs=1) as warm_pool,
            tc.tile_pool(name="fslab", bufs=1) as f_pool,
            tc.tile_pool(name="mslab", bufs=1) as m_pool,
            tc.tile_pool(name="outp", bufs=1) as out_pool,
            tc.tile_pool(name="psum", bufs=4, space="PSUM") as psum_pool,
        ):
            # Warm-up source via VectorE so GpSimd's first op is the SWDGE
            # descriptor generation for feature chunk 1.
            wsrc = warm_pool.tile([P, 512], FP8)
            nc.vector.memset(wsrc[:], 0.25)

            # SP HWDGE ring order: first mask strip, iv, remaining mask
            # strips.  SWDGE (Q7) concurrently generates + streams the
            # feature chunks.
            mask_sb = m_pool.tile([P, m_total], FP8)
            nc.sync.dma_start(
                out=mask_sb[:, mchunks[0][0] : mchunks[0][1]],
                in_=mask_d[:, mchunks[0][0] : mchunks[0][1]],
            )
            iv_sb = iv_pool.tile([P, IVW], F32)
            nc.sync.dma_start(out=iv_sb[:], in_=iv_d[:])
            fhi_tiles = []
            k2chunk = []
            k0 = 0
            for j, sz in enumerate(fchunks):
                fh = f_pool.tile([P, sz, D], FP8, name=f"fh{j}", tag=f"fh{j}")
                eng = nc.sync if j < nsync else nc.gpsimd
                eng.dma_start(out=fh[:], in_=fhi_d[:, k0 : k0 + sz, :])
                fhi_tiles.append(fh)
                for s in range(sz):
                    k2chunk.append((j, s))
                k0 += sz
            assert k0 == nkt

            # Remaining mask strips.
            for lo, hi in mchunks[1:]:
                nc.sync.dma_start(
                    out=mask_sb[:, lo:hi], in_=mask_d[:, lo:hi]
                )

            # PE p-state warm-up: keep the PE busy until the first slot's
            # mask + feature DMAs land.
            wps = psum_pool.tile([P, 1024], F32, name="warm", tag="ps")
            for _ in range(N_WARM):
                nc.tensor.matmul(
                    wps[:, 0:512], wsrc[:, 0:P], wsrc[:], start=True, stop=True
                )

            os_slab = out_pool.tile([P, ns, D], FP8)
            gcuts = np.cumsum([0] + ogroups)
            gi = 0
            for s in range(ns):
                # [P, 1024] = exactly 2 PSUM banks per buf; 4 bufs fill all
                # 8 banks and give the evacuation two slots of slack.
                ps = psum_pool.tile([P, 1024], F32, name=f"ps{s}", tag="ps")
                np_s = len(pairs[s])
                for j, (cb, w, moff, kt) in enumerate(pairs[s]):
                    lh = mask_sb[:, cb : cb + w]
                    cj, cs = k2chunk[kt]
                    rh = fhi_tiles[cj][:, cs, :]
                    first = j == 0
                    last = j == np_s - 1
                    for n0, nn in ((0, 512), (512, 256)):
                        nc.tensor.matmul(
                            ps[moff : moff + w, n0 : n0 + nn],
                            lh, rh[:, n0 : n0 + nn],
                            start=first, stop=(last and n0 == 512),
                        )
                # PSUM evacuation with the 1/count scale; 384+384 balances
                # ScalarE (0.83 ns/el + 293 ns) vs VectorE (1.04 + 217).
                nc.scalar.mul(
                    out=os_slab[:, s, 0:384], in_=ps[:, 0:384],
                    mul=iv_sb[:, s : s + 1],
                )
                nc.vector.tensor_scalar(
                    os_slab[:, s, 384:D], ps[:, 384:D],
                    iv_sb[:, s : s + 1], None, mybir.AluOpType.mult,
                )
                if s == gcuts[gi + 1] - 1:
                    g0, g1 = gcuts[gi], gcuts[gi + 1]
                    r = last_rows if g1 == ns and g1 - g0 == 1 else P
                    nc.sync.dma_start(
                        out=out_d[0:r, g0 * D : g1 * D],
                        in_=os_slab[0:r, g0:g1, :],
                    )
                    gi += 1

    nc.finalize()
    return nc


def _assign_slots(b, e_eff):
    """Per-core greedy slot assignment in sorted-begin order: a slot takes
    up to 128 windows whose token-interval union stays <= SLOT_TOK tokens.

    Returns (ns, slot_of[B,W], pos_of[B,W], slot_tokens[c][s] -> np.array).
    """
    slot_of = np.full((B, W), -1, np.int32)
    pos_of = np.full((B, W), -1, np.int32)
    slot_tokens = []
    ns = 0
    for c in range(B):
        order = np.argsort(b[c], kind="stable")
        bs, es = b[c][order], e_eff[c][order]
        toks_c = []
        i = 0
        while i < W:
            covered = 0
            cur_end = int(bs[i])
            ivals = []
            j = i
            while j < W and j - i < P:
                nb, ne = int(bs[j]), int(es[j])
                add = max(0, ne - max(nb, cur_end))
                if covered + add > SLOT_TOK:
                    break
                covered += add
                if ne > cur_end:
                    ivals.append((max(nb, cur_end), ne))
                    cur_end = ne
                j += 1
            s = len(toks_c)
            slot_of[c, order[i:j]] = s
            pos_of[c, order[i:j]] = np.arange(j - i)
            toks_c.append(
                np.concatenate([np.arange(a, z) for a, z in ivals])
                if ivals
                else np.zeros(0, np.int64)
            )
            i = j
        slot_tokens.append(toks_c)
        ns = max(ns, len(toks_c))
    return ns, slot_of, pos_of, slot_tokens


def _prepare(features, begins, ends):
    feats = np.asarray(features, dtype=np.float32)
    assert feats.shape == (B, T, D), feats.shape
    b = np.clip(np.asarray(begins).astype(np.int64), 0, T - 1)
    e = np.asarray(ends).astype(np.int64)
    # Reference gathers at most MAXWIN tokens starting at b; empty -> count 1.
    e_eff = np.clip(e, b, np.minimum(b + MAXWIN, T))
    counts = np.maximum(e_eff - b, 1).astype(np.float32)
    inv = (1.0 / counts).astype(np.float32)

    ns, slot_of, pos_of, slot_tokens = _assign_slots(b, e_eff)
    assert ns <= IVW, ns

    # Joint (cross-core max) pair structure per slot.  Windows are sorted by
    # begin, so windows starting in tile 0 are a position-prefix [0, c1) and
    # windows reaching into tile 1 are a position-suffix [c0, 128), c0 <= c1.
    # Ship only mask columns [0, c1r) for pair 0 and [c0r, 128) for pair 1
    # (32-aligned); the two matmuls' M-ranges still cover every PSUM row.
    lbs = [[None] * ns for _ in range(B)]
    les = [[None] * ns for _ in range(B)]
    c1s = np.zeros(ns, np.int64)
    c0s = np.full(ns, P, np.int64)
    two = np.zeros(ns, bool)  # slot has any tile-1 tokens on any core
    for c in range(B):
        for s, toks in enumerate(slot_tokens[c]):
            ws = np.nonzero(slot_of[c] == s)[0]
            if not len(ws):
                continue
            o = np.argsort(pos_of[c, ws])
            ws = ws[o]
            lb = np.searchsorted(toks, b[c, ws])
            le = lb + (e_eff[c, ws] - b[c, ws])
            lbs[c][s], les[c][s] = lb, le
            c1s[s] = max(c1s[s], int((lb < P).sum()))
            over = np.nonzero(le > P)[0]
            if len(over):
                c0s[s] = min(c0s[s], int(over[0]))
            if len(toks) > P:
                two[s] = True
    # A matmul PSUM write with a nonzero partition offset may span at most
    # one 32-partition col-group (walrus birverifier), so only pair 0's
    # column TAIL is trimmed (its windows are a position-prefix).  The
    # full-width tile-1 pair runs FIRST in each slot: its start=True write
    # covers (and has_written-clears) every PSUM row, and the trimmed
    # tile-0 pair then accumulates on rows [0, c1r) only.
    pairs = []
    kt = 0
    col = 0
    for s in range(ns):
        if two[s]:
            c1r = min(P, -(-int(c1s[s]) // 32) * 32)
            pairs.append([(col, P, 0, kt + 1), (col + P, c1r, 0, kt)])
            col += P + c1r
            kt += 2
        else:
            pairs.append([(col, P, 0, kt)])  # full width: writes every row
            col += P
            kt += 1
    nkt, m_total = kt, col

    in_maps = []
    unperm = []
    for c in range(B):
        hi = np.zeros((P, nkt, D), NP_FP8)
        slab = np.zeros((P, m_total), NP_FP8)
        ivm = np.zeros((P, IVW), np.float32)
        ivm[pos_of[c], slot_of[c]] = inv[c]
        for s, toks in enumerate(slot_tokens[c]):
            nt = len(toks)
            ktiles = sorted(pr[3] for pr in pairs[s])
            if nt:
                pk = feats[c, toks].astype(NP_FP8)
                pad = np.zeros((P * len(ktiles), D), NP_FP8)
                pad[:nt] = pk
                hi[:, ktiles, :] = pad.reshape(len(ktiles), P, D).transpose(
                    1, 0, 2
                )
            if lbs[c][s] is None:
                continue
            lb, le = lbs[c][s], les[c][s]
            nw = len(lb)
            for cb, w, moff, pkt in pairs[s]:
                tloc = pkt - ktiles[0]  # which 128-token tile of the slot
                lt = P * tloc + np.arange(P)  # local token row per partition
                sub = (
                    (lb[None, :] <= lt[:, None]) & (lt[:, None] < le[None, :])
                ).astype(NP_FP8)
                # columns for positions [moff, moff+w) of this slot
                pcols = np.arange(moff, min(moff + w, nw))
                slab[:, cb + pcols - moff] = sub[:, pcols]
        in_maps.append({"fhi": hi, "mask": slab, "iv": ivm})
        unperm.append((slot_of[c].astype(np.int64), pos_of[c].astype(np.int64)))
    last_rows = max(
        1, int((slot_of == ns - 1).sum(axis=1).max())
    )
    return ns, nkt, pairs, m_total, in_maps, unperm, last_rows


def run(features, begins, ends, trace=False):
    """Build + run on 8 NeuronCores; returns (output, BassKernelResults)."""
    ns, nkt, pairs, m_total, in_maps, unperm, last_rows = _prepare(
        features, begins, ends
    )
    nc = _build_program(ns, nkt, pairs, m_total, last_rows)
    res = run_bass_kernel_spmd(nc, in_maps, list(range(B)), trace=trace)
    out = np.stack(
        [
            res.results[c]["out"]
            .reshape(P, ns, D)[unperm[c][1], unperm[c][0]]
            .astype(np.float32)
            for c in range(B)
        ],
        axis=0,
    )
    return out, res


def kernel(features, begins, ends):
    out, _ = run(features, begins, ends, trace=False)
    return out


# revision 48
# speedup vs baseline: 1.1808x; 1.0550x over previous
"""Trainium2 Bass kernel for windowed mean-pooling (segment_reduce).

Computes, for each (batch b, window w):
    out[b, w, :] = mean over t in [begins[b,w], ends'[b,w]) of features[b, t, :]
where ends' = clip(ends, begins, begins + 8) (the reference gathers at most
MAX_WINDOW=8 tokens) and empty windows produce 0 (count clamped to >= 1).

Strategy (data-parallel over batch, one sample per NeuronCore):
  - Windows are mean-pooled via mask matmuls: out_slot = M^T @ F with M a
    host-built fp8 0/1 strip, F the slot's tokens, fp32 PSUM accumulate.
  - Windows are assigned to SLOTS greedily in sorted-begin order: a slot
    takes up to 128 windows as long as the union of their token intervals
    fits in 256 tokens.  The slot's tokens are RE-PACKED on host into
    exactly 2 aligned K-tiles, so every slot costs exactly 2 (slot, K-tile)
    mask matmul pairs (~33 total vs ~49 for global aligned packing); slots
    whose tokens fit one tile (the runt) cost a single pair.
  - The kernel is HBM-byte-bound (~5.3 MB/core ~ 15 us at 358 GB/s), so
    bytes are minimized hard: features fp8 E3M4 (~3.2 MB incl. repack
    padding), masks fp8 with the tile-0 strip's zero column-tail trimmed
    (the full-width tile-1 strip runs FIRST so its start=True write
    has_written-covers every PSUM row; the trimmed strip then accumulates),
    outputs fp8 in a partition-major [P, ns*D] layout (contiguous multi-KB
    descriptors), fp32 1/count scales padded to 512 B/partition.
  - PSUM evacuation applies the per-window 1/count scale, split 384+384
    across ScalarE ((N+352)/1.2 ns) and VectorE ((N+~210)/0.96 ns), which
    balance at that split.  PSUM tiles are [P, 1024] = exactly 2 banks so
    no two in-flight slots share a bank (a shared bank serializes the DVE
    read of slot s against slot s+1's PE writes); 4 bufs fill all 8 banks.
  - Startup: warm-up matmuls hold the PE's HAM activity window open while
    the first mask strip (SP HWDGE ring) and feature chunks (GPSIMD SWDGE,
    growing chunk sizes) land; the warm-up source comes from a VectorE
    memset so GpSimd's first op is SWDGE descriptor generation.  Putting
    feature chunks on the SP ring ahead of the masks measured consistently
    worse -- masks gate the same matmuls and the SP ring serializes.
"""

import os
import sys

import numpy as np

for _p in ("/opt/trn_rl_repo", "/root/.axon_site/_ro/trn_rl_repo"):
    if os.path.isdir(_p) and _p not in sys.path:
        sys.path.insert(0, _p)

from concourse import bacc, mybir  # noqa: E402
import concourse.tile as tile  # noqa: E402
from concourse.bass_utils import run_bass_kernel_spmd  # noqa: E402

B, T, D, W = 8, 4096, 768, 2048
MAXWIN = 8
P = 128
SLOT_TOK = 256  # tokens per slot (2 K-tiles, repacked)
N_WARM = 6  # PE p-state warm-up matmuls bridging until the first DMAs land
IVSLOT = 8  # the fp32 1/count block is embedded in the mask slab just
# before this slot's strips, so it arrives with the second mask chunk
F32 = mybir.dt.float32
FP8 = mybir.dt.float8e3
NP_FP8 = mybir.dt.np(mybir.dt.float8e3)


def _fchunks(n, sizes0=(2, 4, 6), big=8):
    """Chunk sizes, small first so early slots' data lands first."""
    sizes = []
    for s in sizes0:
        if sum(sizes) + s > n:
            break
        sizes.append(s)
    rem = n - sum(sizes)
    while rem > 0:
        take = min(big, rem)
        sizes.append(take)
        rem -= take
    assert sum(sizes) == n and all(s > 0 for s in sizes), (sizes, n)
    return sizes


def _ogroups(ns):
    """Output DMA slot groups; small tail groups shorten the drain."""
    groups = []
    rem = ns
    while rem > 3:
        take = min(5, rem - 2)
        groups.append(take)
        rem -= take
    if rem > 1:
        groups.append(rem - 1)
        rem = 1
    groups.append(1)
    assert sum(groups) == ns
    return groups


def _build_program(ns, nkt, pairs, m_total, ivbase, last_rows=P):
    """pairs: per slot, list of (mask col base, width, psum row offset,
    feature tile index).  ivbase: column of the fp32 1/count block embedded
    in the mask slab (read via bitcast).  last_rows: used rows of the final
    (runt) slot -- its out DMA ships only those partitions."""
    nc = bacc.Bacc(None)

    fhi_d = nc.declare_dram_parameter("fhi", [P, nkt, D], FP8, isOutput=False)
    mask_d = nc.declare_dram_parameter("mask", [P, m_total], FP8, isOutput=False)
    out_d = nc.declare_dram_parameter("out", [P, ns * D], FP8, isOutput=True)

    # Feature chunks stream via GPSIMD SWDGE; masks (with the embedded
    # 1/count block) ride the SP HWDGE ring.
    # Chunk sizes [2,4,6,8,8,...]: coarser chunks measured better than
    # fine-grained ones -- each SWDGE dma_start pays its own ~2-3 us
    # completion receipt, and a deeper SWDGE queue makes receipts worse.
    fchunks = _fchunks(nkt)
    nsync = 0  # features all via SWDGE: measured faster than any mix that
    # puts feature chunks ahead of the mask strips on the SP ring
    # Mask chunk cuts at slot-strip starts; the iv block sits just before
    # slot IVSLOT's strips so it arrives with the second chunk.
    cutcols = [0] + [
        pairs[s][0][0] for s in (2, IVSLOT) if s < ns
    ] + [m_total]
    cutcols = sorted(set(cutcols))
    mchunks = list(zip(cutcols[:-1], cutcols[1:]))
    ogroups = _ogroups(ns)

    with tile.TileContext(nc) as tc:
        with (
            tc.tile_pool(name="warm", bufs=1) as warm_pool,
            tc.tile_pool(name="fslab", bufs=1) as f_pool,
            tc.tile_pool(name="mslab", bufs=1) as m_pool,
            tc.tile_pool(name="outp", bufs=1) as out_pool,
            tc.tile_pool(name="psum", bufs=4, space="PSUM") as psum_pool,
        ):
            # Warm-up source via VectorE so GpSimd's first op is the SWDGE
            # descriptor generation for feature chunk 1.
            wsrc = warm_pool.tile([P, 512], FP8)
            nc.vector.memset(wsrc[:], 0.25)

            # SP HWDGE ring order: first mask strip, then the remaining
            # strips (the second carries the embedded 1/count block).
            # SWDGE (Q7) concurrently generates + streams feature chunks.
            mask_sb = m_pool.tile([P, m_total], FP8)
            nc.sync.dma_start(
                out=mask_sb[:, mchunks[0][0] : mchunks[0][1]],
                in_=mask_d[:, mchunks[0][0] : mchunks[0][1]],
            )
            fhi_tiles = []
            k2chunk = []
            k0 = 0
            for j, sz in enumerate(fchunks):
                fh = f_pool.tile([P, sz, D], FP8, name=f"fh{j}", tag=f"fh{j}")
                eng = nc.sync if j < nsync else nc.gpsimd
                eng.dma_start(out=fh[:], in_=fhi_d[:, k0 : k0 + sz, :])
                fhi_tiles.append(fh)
                for s in range(sz):
                    k2chunk.append((j, s))
                k0 += sz
            assert k0 == nkt

            # Remaining mask strips.
            for lo, hi in mchunks[1:]:
                nc.sync.dma_start(
                    out=mask_sb[:, lo:hi], in_=mask_d[:, lo:hi]
                )

            # PE p-state warm-up: keep the PE busy until the first slot's
            # mask + feature DMAs land.
            wps = psum_pool.tile([P, 1024], F32, name="warm", tag="ps")
            for _ in range(N_WARM):
                nc.tensor.matmul(
                    wps[:, 0:512], wsrc[:, 0:P], wsrc[:], start=True, stop=True
                )

            os_slab = out_pool.tile([P, ns, D], FP8)
            gcuts = np.cumsum([0] + ogroups)
            gi = 0
            for s in range(ns):
                # [P, 1024] = exactly 2 PSUM banks per buf; 4 bufs fill all
                # 8 banks and give the evacuation two slots of slack.
                ps = psum_pool.tile([P, 1024], F32, name=f"ps{s}", tag="ps")
                np_s = len(pairs[s])
                for j, (cb, w, moff, kt) in enumerate(pairs[s]):
                    lh = mask_sb[:, cb : cb + w]
                    cj, cs = k2chunk[kt]
                    rh = fhi_tiles[cj][:, cs, :]
                    first = j == 0
                    last = j == np_s - 1
                    for n0, nn in ((0, 512), (512, 256)):
                        nc.tensor.matmul(
                            ps[moff : moff + w, n0 : n0 + nn],
                            lh, rh[:, n0 : n0 + nn],
                            start=first, stop=(last and n0 == 512),
                        )
                # PSUM evacuation with the 1/count scale (read out of the
                # mask slab via a 4-byte fp8->fp32 bitcast); 384+384 balances
                # ScalarE (0.83 ns/el + 293 ns) vs VectorE (1.04 + 217).
                iv_s = mask_sb[:, ivbase + 4 * s : ivbase + 4 * s + 4].bitcast(
                    F32
                )
                nc.scalar.mul(
                    out=os_slab[:, s, 0:384], in_=ps[:, 0:384], mul=iv_s
                )
                nc.vector.tensor_scalar(
                    os_slab[:, s, 384:D], ps[:, 384:D],
                    iv_s, None, mybir.AluOpType.mult,
                )
                if s == gcuts[gi + 1] - 1:
                    g0, g1 = gcuts[gi], gcuts[gi + 1]
                    r = last_rows if g1 == ns and g1 - g0 == 1 else P
                    nc.sync.dma_start(
                        out=out_d[0:r, g0 * D : g1 * D],
                        in_=os_slab[0:r, g0:g1, :],
                    )
                    gi += 1

    nc.finalize()
    return nc


def _assign_slots(b, e_eff):
    """Per-core greedy slot assignment in sorted-begin order: a slot takes
    up to 128 windows whose token-interval union stays <= SLOT_TOK tokens.

    Returns (ns, slot_of[B,W], pos_of[B,W], slot_tokens[c][s] -> np.array).
    """
    slot_of = np.full((B, W), -1, np.int32)
    pos_of = np.full((B, W), -1, np.int32)
    slot_tokens = []
    ns = 0
    for c in range(B):
        order = np.argsort(b[c], kind="stable")
        bs, es = b[c][order], e_eff[c][order]
        toks_c = []
        i = 0
        while i < W:
            covered = 0
            cur_end = int(bs[i])
            ivals = []
            j = i
            while j < W and j - i < P:
                nb, ne = int(bs[j]), int(es[j])
                add = max(0, ne - max(nb, cur_end))
                if covered + add > SLOT_TOK:
                    break
                covered += add
                if ne > cur_end:
                    ivals.append((max(nb, cur_end), ne))
                    cur_end = ne
                j += 1
            s = len(toks_c)
            slot_of[c, order[i:j]] = s
            pos_of[c, order[i:j]] = np.arange(j - i)
            toks_c.append(
                np.concatenate([np.arange(a, z) for a, z in ivals])
                if ivals
                else np.zeros(0, np.int64)
            )
            i = j
        slot_tokens.append(toks_c)
        ns = max(ns, len(toks_c))
    return ns, slot_of, pos_of, slot_tokens


def _prepare(features, begins, ends):
    feats = np.asarray(features, dtype=np.float32)
    assert feats.shape == (B, T, D), feats.shape
    b = np.clip(np.asarray(begins).astype(np.int64), 0, T - 1)
    e = np.asarray(ends).astype(np.int64)
    # Reference gathers at most MAXWIN tokens starting at b; empty -> count 1.
    e_eff = np.clip(e, b, np.minimum(b + MAXWIN, T))
    counts = np.maximum(e_eff - b, 1).astype(np.float32)
    inv = (1.0 / counts).astype(np.float32)

    ns, slot_of, pos_of, slot_tokens = _assign_slots(b, e_eff)

    # Joint (cross-core max) pair structure per slot.  Windows are sorted by
    # begin, so windows starting in tile 0 are a position-prefix [0, c1) and
    # windows reaching into tile 1 are a position-suffix [c0, 128), c0 <= c1.
    # Ship only mask columns [0, c1r) for pair 0 and [c0r, 128) for pair 1
    # (32-aligned); the two matmuls' M-ranges still cover every PSUM row.
    lbs = [[None] * ns for _ in range(B)]
    les = [[None] * ns for _ in range(B)]
    c1s = np.zeros(ns, np.int64)
    c0s = np.full(ns, P, np.int64)
    two = np.zeros(ns, bool)  # slot has any tile-1 tokens on any core
    for c in range(B):
        for s, toks in enumerate(slot_tokens[c]):
            ws = np.nonzero(slot_of[c] == s)[0]
            if not len(ws):
                continue
            o = np.argsort(pos_of[c, ws])
            ws = ws[o]
            lb = np.searchsorted(toks, b[c, ws])
            le = lb + (e_eff[c, ws] - b[c, ws])
            lbs[c][s], les[c][s] = lb, le
            c1s[s] = max(c1s[s], int((lb < P).sum()))
            over = np.nonzero(le > P)[0]
            if len(over):
                c0s[s] = min(c0s[s], int(over[0]))
            if len(toks) > P:
                two[s] = True
    # A matmul PSUM write with a nonzero partition offset may span at most
    # one 32-partition col-group (walrus birverifier), so only pair 0's
    # column TAIL is trimmed (its windows are a position-prefix).  The
    # full-width tile-1 pair runs FIRST in each slot: its start=True write
    # covers (and has_written-clears) every PSUM row, and the trimmed
    # tile-0 pair then accumulates on rows [0, c1r) only.
    pairs = []
    kt = 0
    col = 0
    ivblk = -(-4 * ns // 32) * 32
    ivbase = -1
    for s in range(ns):
        if s == IVSLOT:
            ivbase = col
            col += ivblk
        if two[s]:
            c1r = min(P, -(-int(c1s[s]) // 32) * 32)
            pairs.append([(col, P, 0, kt + 1), (col + P, c1r, 0, kt)])
            col += P + c1r
            kt += 2
        else:
            pairs.append([(col, P, 0, kt)])  # full width: writes every row
            col += P
            kt += 1
    if ivbase < 0:
        ivbase = col
        col += ivblk
    nkt, m_total = kt, col

    in_maps = []
    unperm = []
    for c in range(B):
        hi = np.zeros((P, nkt, D), NP_FP8)
        slab = np.zeros((P, m_total), NP_FP8)
        ivm = np.zeros((P, ns), np.float32)
        ivm[pos_of[c], slot_of[c]] = inv[c]
        slab[:, ivbase : ivbase + 4 * ns] = ivm.view(NP_FP8)
        for s, toks in enumerate(slot_tokens[c]):
            nt = len(toks)
            ktiles = sorted(pr[3] for pr in pairs[s])
            if nt:
                pk = feats[c, toks].astype(NP_FP8)
                pad = np.zeros((P * len(ktiles), D), NP_FP8)
                pad[:nt] = pk
                hi[:, ktiles, :] = pad.reshape(len(ktiles), P, D).transpose(
                    1, 0, 2
                )
            if lbs[c][s] is None:
                continue
            lb, le = lbs[c][s], les[c][s]
            nw = len(lb)
            for cb, w, moff, pkt in pairs[s]:
                tloc = pkt - ktiles[0]  # which 128-token tile of the slot
                lt = P * tloc + np.arange(P)  # local token row per partition
                sub = (
                    (lb[None, :] <= lt[:, None]) & (lt[:, None] < le[None, :])
                ).astype(NP_FP8)
                # columns for positions [moff, moff+w) of this slot
                pcols = np.arange(moff, min(moff + w, nw))
                slab[:, cb + pcols - moff] = sub[:, pcols]
        in_maps.append({"fhi": hi, "mask": slab})
        unperm.append((slot_of[c].astype(np.int64), pos_of[c].astype(np.int64)))
    last_rows = max(
        1, int((slot_of == ns - 1).sum(axis=1).max())
    )
    return ns, nkt, pairs, m_total, ivbase, in_maps, unperm, last_rows


def run(features, begins, ends, trace=False):
    """Build + run on 8 NeuronCores; returns (output, BassKernelResults)."""
    ns, nkt, pairs, m_total, ivbase, in_maps, unperm, last_rows = _prepare(
        features, begins, ends
    )
    nc = _build_program(ns, nkt, pairs, m_total, ivbase, last_rows)
    res = run_bass_kernel_spmd(nc, in_maps, list(range(B)), trace=trace)
    out = np.stack(
        [
            res.results[c]["out"]
            .reshape(P, ns, D)[unperm[c][1], unperm[c][0]]
            .astype(np.float32)
            for c in range(B)
        ],
        axis=0,
    )
    return out, res


def kernel(features, begins, ends):
    out, _ = run(features, begins, ends, trace=False)
    return out


# revision 49
# speedup vs baseline: 1.2082x; 1.0232x over previous
"""Trainium2 Bass kernel for windowed mean-pooling (segment_reduce).

Computes, for each (batch b, window w):
    out[b, w, :] = mean over t in [begins[b,w], ends'[b,w]) of features[b, t, :]
where ends' = clip(ends, begins, begins + 8) (the reference gathers at most
MAX_WINDOW=8 tokens) and empty windows produce 0 (count clamped to >= 1).

Strategy (data-parallel over batch, one sample per NeuronCore):
  - Windows are mean-pooled via mask matmuls: out_slot = M^T @ F with M a
    host-built fp8 0/1 strip, F the slot's tokens, fp32 PSUM accumulate.
  - Windows are assigned to SLOTS greedily in sorted-begin order: a slot
    takes up to 128 windows as long as the union of their token intervals
    fits in 256 tokens.  The slot's tokens are RE-PACKED on host into
    exactly 2 aligned K-tiles, so every slot costs exactly 2 (slot, K-tile)
    mask matmul pairs (~33 total vs ~49 for global aligned packing); slots
    whose tokens fit one tile (the runt) cost a single pair.
  - The kernel is HBM-byte-bound (~5.3 MB/core ~ 15 us at 358 GB/s), so
    bytes are minimized hard: features fp8 E3M4 (~3.2 MB incl. repack
    padding), masks fp8 with the tile-0 strip's zero column-tail trimmed
    (the full-width tile-1 strip runs FIRST so its start=True write
    has_written-covers every PSUM row; the trimmed strip then accumulates),
    outputs fp8 in a partition-major [P, ns*D] layout (contiguous multi-KB
    descriptors).  The fp32 1/count scales are EMBEDDED in the mask slab
    (read back via a 4-byte fp8->fp32 bitcast) -- one fewer dma_start on
    the startup-critical SP ring saves its ~0.65 us descriptor-generation
    and ~2 us completion receipt, worth ~2 us end-to-end (measured).
  - PSUM evacuation applies the per-window 1/count scale, split 384+384
    across ScalarE ((N+352)/1.2 ns) and VectorE ((N+~210)/0.96 ns), which
    balance at that split.  PSUM tiles are [P, 1024] = exactly 2 banks so
    no two in-flight slots share a bank (a shared bank serializes the DVE
    read of slot s against slot s+1's PE writes); 4 bufs fill all 8 banks.
  - Startup: warm-up matmuls hold the PE's HAM activity window open while
    the first mask strip (SP HWDGE ring) and feature chunks (GPSIMD SWDGE,
    growing chunk sizes) land; the warm-up source comes from a VectorE
    memset so GpSimd's first op is SWDGE descriptor generation.  Putting
    feature chunks on the SP ring ahead of the masks measured consistently
    worse -- masks gate the same matmuls and the SP ring serializes.
"""

import os
import sys

import numpy as np

for _p in ("/opt/trn_rl_repo", "/root/.axon_site/_ro/trn_rl_repo"):
    if os.path.isdir(_p) and _p not in sys.path:
        sys.path.insert(0, _p)

from concourse import bacc, mybir  # noqa: E402
import concourse.tile as tile  # noqa: E402
from concourse.bass_utils import run_bass_kernel_spmd  # noqa: E402

B, T, D, W = 8, 4096, 768, 2048
MAXWIN = 8
P = 128
SLOT_TOK = 256  # tokens per slot (2 K-tiles, repacked)
N_WARM = 6  # PE p-state warm-up matmuls bridging until the first DMAs land
IVSLOT = 8  # the fp32 1/count block is embedded in the mask slab just
# before this slot's strips, so it arrives with the second mask chunk
F32 = mybir.dt.float32
FP8 = mybir.dt.float8e3
NP_FP8 = mybir.dt.np(mybir.dt.float8e3)


def _fchunks(n, sizes0=(2, 4, 6), big=8):
    """Chunk sizes, small first so early slots' data lands first."""
    sizes = []
    for s in sizes0:
        if sum(sizes) + s > n:
            break
        sizes.append(s)
    rem = n - sum(sizes)
    while rem > 0:
        take = min(big, rem)
        sizes.append(take)
        rem -= take
    assert sum(sizes) == n and all(s > 0 for s in sizes), (sizes, n)
    return sizes


def _ogroups(ns):
    """Output DMA slot groups; small tail groups shorten the drain."""
    groups = []
    rem = ns
    while rem > 3:
        take = min(5, rem - 2)
        groups.append(take)
        rem -= take
    if rem > 1:
        groups.append(rem - 1)
        rem = 1
    groups.append(1)
    assert sum(groups) == ns
    return groups


def _build_program(ns, nkt, pairs, m_total, ivbase, last_rows=P):
    """pairs: per slot, list of (mask col base, width, psum row offset,
    feature tile index).  ivbase: column of the fp32 1/count block embedded
    in the mask slab (read via bitcast).  last_rows: used rows of the final
    (runt) slot -- its out DMA ships only those partitions."""
    nc = bacc.Bacc(None)

    fhi_d = nc.declare_dram_parameter("fhi", [P, nkt, D], FP8, isOutput=False)
    mask_d = nc.declare_dram_parameter("mask", [P, m_total], FP8, isOutput=False)
    out_d = nc.declare_dram_parameter("out", [P, ns * D], FP8, isOutput=True)

    # Feature chunks stream via GPSIMD SWDGE; masks (with the embedded
    # 1/count block) ride the SP HWDGE ring.
    # Chunk sizes [2,4,6,8,8,...]: coarser chunks measured better than
    # fine-grained ones -- each SWDGE dma_start pays its own ~2-3 us
    # completion receipt, and a deeper SWDGE queue makes receipts worse.
    fchunks = _fchunks(nkt)
    nsync = 0  # features all via SWDGE: measured faster than any mix that
    # puts feature chunks ahead of the mask strips on the SP ring
    # Mask chunk cuts at slot-strip starts; the iv block sits just before
    # slot IVSLOT's strips so it arrives with the second chunk.
    cutcols = [0] + [
        pairs[s][0][0] for s in (2, IVSLOT) if s < ns
    ] + [m_total]
    cutcols = sorted(set(cutcols))
    mchunks = list(zip(cutcols[:-1], cutcols[1:]))
    ogroups = _ogroups(ns)

    with tile.TileContext(nc) as tc:
        with (
            tc.tile_pool(name="warm", bufs=1) as warm_pool,
            tc.tile_pool(name="fslab", bufs=1) as f_pool,
            tc.tile_pool(name="mslab", bufs=1) as m_pool,
            tc.tile_pool(name="outp", bufs=1) as out_pool,
            tc.tile_pool(name="psum", bufs=4, space="PSUM") as psum_pool,
        ):
            # Warm-up source via VectorE so GpSimd's first op is the SWDGE
            # descriptor generation for feature chunk 1.
            wsrc = warm_pool.tile([P, 512], FP8)
            nc.vector.memset(wsrc[:], 0.25)

            # SP HWDGE ring order: first mask strip, then the remaining
            # strips (the second carries the embedded 1/count block).
            # SWDGE (Q7) concurrently generates + streams feature chunks.
            mask_sb = m_pool.tile([P, m_total], FP8)
            nc.sync.dma_start(
                out=mask_sb[:, mchunks[0][0] : mchunks[0][1]],
                in_=mask_d[:, mchunks[0][0] : mchunks[0][1]],
            )
            fhi_tiles = []
            k2chunk = []
            k0 = 0
            for j, sz in enumerate(fchunks):
                fh = f_pool.tile([P, sz, D], FP8, name=f"fh{j}", tag=f"fh{j}")
                eng = nc.sync if j < nsync else nc.gpsimd
                eng.dma_start(out=fh[:], in_=fhi_d[:, k0 : k0 + sz, :])
                fhi_tiles.append(fh)
                for s in range(sz):
                    k2chunk.append((j, s))
                k0 += sz
            assert k0 == nkt

            # Remaining mask strips.
            for lo, hi in mchunks[1:]:
                nc.sync.dma_start(
                    out=mask_sb[:, lo:hi], in_=mask_d[:, lo:hi]
                )

            # PE p-state warm-up: keep the PE busy until the first slot's
            # mask + feature DMAs land.
            wps = psum_pool.tile([P, 1024], F32, name="warm", tag="ps")
            for _ in range(N_WARM):
                nc.tensor.matmul(
                    wps[:, 0:512], wsrc[:, 0:P], wsrc[:], start=True, stop=True
                )

            os_slab = out_pool.tile([P, ns, D], FP8)
            gcuts = np.cumsum([0] + ogroups)
            gi = 0
            for s in range(ns):
                # [P, 1024] = exactly 2 PSUM banks per buf; 4 bufs fill all
                # 8 banks and give the evacuation two slots of slack.
                ps = psum_pool.tile([P, 1024], F32, name=f"ps{s}", tag="ps")
                np_s = len(pairs[s])
                for j, (cb, w, moff, kt) in enumerate(pairs[s]):
                    lh = mask_sb[:, cb : cb + w]
                    cj, cs = k2chunk[kt]
                    rh = fhi_tiles[cj][:, cs, :]
                    first = j == 0
                    last = j == np_s - 1
                    for n0, nn in ((0, 512), (512, 256)):
                        nc.tensor.matmul(
                            ps[moff : moff + w, n0 : n0 + nn],
                            lh, rh[:, n0 : n0 + nn],
                            start=first, stop=(last and n0 == 512),
                        )
                # PSUM evacuation with the 1/count scale (read out of the
                # mask slab via a 4-byte fp8->fp32 bitcast); 384+384 balances
                # ScalarE (0.83 ns/el + 293 ns) vs VectorE (1.04 + 217).
                iv_s = mask_sb[:, ivbase + 4 * s : ivbase + 4 * s + 4].bitcast(
                    F32
                )
                nc.scalar.mul(
                    out=os_slab[:, s, 0:384], in_=ps[:, 0:384], mul=iv_s
                )
                nc.vector.tensor_scalar(
                    os_slab[:, s, 384:D], ps[:, 384:D],
                    iv_s, None, mybir.AluOpType.mult,
                )
                if s == gcuts[gi + 1] - 1:
                    g0, g1 = gcuts[gi], gcuts[gi + 1]
                    r = last_rows if g1 == ns and g1 - g0 == 1 else P
                    nc.sync.dma_start(
                        out=out_d[0:r, g0 * D : g1 * D],
                        in_=os_slab[0:r, g0:g1, :],
                    )
                    gi += 1

    nc.finalize()
    return nc


def _assign_slots(b, e_eff):
    """Per-core greedy slot assignment in sorted-begin order: a slot takes
    up to 128 windows whose token-interval union stays <= SLOT_TOK tokens.

    Returns (ns, slot_of[B,W], pos_of[B,W], slot_tokens[c][s] -> np.array).
    """
    slot_of = np.full((B, W), -1, np.int32)
    pos_of = np.full((B, W), -1, np.int32)
    slot_tokens = []
    ns = 0
    for c in range(B):
        order = np.argsort(b[c], kind="stable")
        bs, es = b[c][order], e_eff[c][order]
        toks_c = []
        i = 0
        while i < W:
            covered = 0
            cur_end = int(bs[i])
            ivals = []
            j = i
            while j < W and j - i < P:
                nb, ne = int(bs[j]), int(es[j])
                add = max(0, ne - max(nb, cur_end))
                if covered + add > SLOT_TOK:
                    break
                covered += add
                if ne > cur_end:
                    ivals.append((max(nb, cur_end), ne))
                    cur_end = ne
                j += 1
            s = len(toks_c)
            slot_of[c, order[i:j]] = s
            pos_of[c, order[i:j]] = np.arange(j - i)
            toks_c.append(
                np.concatenate([np.arange(a, z) for a, z in ivals])
                if ivals
                else np.zeros(0, np.int64)
            )
            i = j
        slot_tokens.append(toks_c)
        ns = max(ns, len(toks_c))
    return ns, slot_of, pos_of, slot_tokens


def _prepare(features, begins, ends):
    feats = np.asarray(features, dtype=np.float32)
    assert feats.shape == (B, T, D), feats.shape
    b = np.clip(np.asarray(begins).astype(np.int64), 0, T - 1)
    e = np.asarray(ends).astype(np.int64)
    # Reference gathers at most MAXWIN tokens starting at b; empty -> count 1.
    e_eff = np.clip(e, b, np.minimum(b + MAXWIN, T))
    counts = np.maximum(e_eff - b, 1).astype(np.float32)
    inv = (1.0 / counts).astype(np.float32)

    ns, slot_of, pos_of, slot_tokens = _assign_slots(b, e_eff)

    # Joint (cross-core max) pair structure per slot.  Windows are sorted by
    # begin, so windows starting in tile 0 are a position-prefix [0, c1) and
    # windows reaching into tile 1 are a position-suffix [c0, 128), c0 <= c1.
    # Ship only mask columns [0, c1r) for pair 0 and [c0r, 128) for pair 1
    # (32-aligned); the two matmuls' M-ranges still cover every PSUM row.
    lbs = [[None] * ns for _ in range(B)]
    les = [[None] * ns for _ in range(B)]
    c1s = np.zeros(ns, np.int64)
    c0s = np.full(ns, P, np.int64)
    two = np.zeros(ns, bool)  # slot has any tile-1 tokens on any core
    for c in range(B):
        for s, toks in enumerate(slot_tokens[c]):
            ws = np.nonzero(slot_of[c] == s)[0]
            if not len(ws):
                continue
            o = np.argsort(pos_of[c, ws])
            ws = ws[o]
            lb = np.searchsorted(toks, b[c, ws])
            le = lb + (e_eff[c, ws] - b[c, ws])
            lbs[c][s], les[c][s] = lb, le
            c1s[s] = max(c1s[s], int((lb < P).sum()))
            over = np.nonzero(le > P)[0]
            if len(over):
                c0s[s] = min(c0s[s], int(over[0]))
            if len(toks) > P:
                two[s] = True
    # A matmul PSUM write with a nonzero partition offset may span at most
    # one 32-partition col-group (walrus birverifier), so only pair 0's
    # column TAIL is trimmed (its windows are a position-prefix).  The
    # full-width tile-1 pair runs FIRST in each slot: its start=True write
    # covers (and has_written-clears) every PSUM row, and the trimmed
    # tile-0 pair then accumulates on rows [0, c1r) only.
    pairs = []
    kt = 0
    col = 0
    ivblk = -(-4 * ns // 32) * 32
    ivbase = -1
    for s in range(ns):
        if s == IVSLOT:
            ivbase = col
            col += ivblk
        if two[s]:
            c1r = min(P, -(-int(c1s[s]) // 32) * 32)
            pairs.append([(col, P, 0, kt + 1), (col + P, c1r, 0, kt)])
            col += P + c1r
            kt += 2
        else:
            pairs.append([(col, P, 0, kt)])  # full width: writes every row
            col += P
            kt += 1
    if ivbase < 0:
        ivbase = col
        col += ivblk
    nkt, m_total = kt, col

    in_maps = []
    unperm = []
    for c in range(B):
        hi = np.zeros((P, nkt, D), NP_FP8)
        slab = np.zeros((P, m_total), NP_FP8)
        ivm = np.zeros((P, ns), np.float32)
        ivm[pos_of[c], slot_of[c]] = inv[c]
        slab[:, ivbase : ivbase + 4 * ns] = ivm.view(NP_FP8)
        for s, toks in enumerate(slot_tokens[c]):
            nt = len(toks)
            ktiles = sorted(pr[3] for pr in pairs[s])
            if nt:
                pk = feats[c, toks].astype(NP_FP8)
                pad = np.zeros((P * len(ktiles), D), NP_FP8)
                pad[:nt] = pk
                hi[:, ktiles, :] = pad.reshape(len(ktiles), P, D).transpose(
                    1, 0, 2
                )
            if lbs[c][s] is None:
                continue
            lb, le = lbs[c][s], les[c][s]
            nw = len(lb)
            for cb, w, moff, pkt in pairs[s]:
                tloc = pkt - ktiles[0]  # which 128-token tile of the slot
                lt = P * tloc + np.arange(P)  # local token row per partition
                sub = (
                    (lb[None, :] <= lt[:, None]) & (lt[:, None] < le[None, :])
                ).astype(NP_FP8)
                # columns for positions [moff, moff+w) of this slot
                pcols = np.arange(moff, min(moff + w, nw))
                slab[:, cb + pcols - moff] = sub[:, pcols]
        in_maps.append({"fhi": hi, "mask": slab})
        unperm.append((slot_of[c].astype(np.int64), pos_of[c].astype(np.int64)))
    last_rows = max(
        1, int((slot_of == ns - 1).sum(axis=1).max())
    )
    return ns, nkt, pairs, m_total, ivbase, in_maps, unperm, last_rows


def run(features, begins, ends, trace=False):
    """Build + run on 8 NeuronCores; returns (output, BassKernelResults)."""
    ns, nkt, pairs, m_total, ivbase, in_maps, unperm, last_rows = _prepare(
        features, begins, ends
    )
    nc = _build_program(ns, nkt, pairs, m_total, ivbase, last_rows)
    res = run_bass_kernel_spmd(nc, in_maps, list(range(B)), trace=trace)
    out = np.stack(
        [
            res.results[c]["out"]
            .reshape(P, ns, D)[unperm[c][1], unperm[c][0]]
            .astype(np.float32)
            for c in range(B)
        ],
        axis=0,
    )
    return out, res


def kernel(features, begins, ends):
    out, _ = run(features, begins, ends, trace=False)
    return out
